# revision 33
# baseline (speedup 1.0000x reference)
"""GAT 2-layer node classification on 8 Trainium2 NeuronCores.

Strategy (self-contained; shapes hardcoded for the fixed problem):
  - Host: add self-loops, sort edges by dst, shard dst nodes contiguously
    across 8 cores, build per-core gather index arrays (int16, split-table
    trick for N>32767). Edge slots per (tile, zone) are packed by dst-OCTANT
    (32-node windows) at 16-slot granularity with exact gather counts, so
    segment softmax/sum run as 32-wide windowed one-hot matmuls:
      * Pm32 [slot, 32] generated on-device (DVE is_equal from ds_rel pairs)
      * PT32 [32, slot] loaded as f8 (32-partition DMA, 4x cheaper than full)
      * a_d per edge via K=32 matmul against host-pre-sliced ad_oct rows
      * a_s folded into the score PSUM via an identity-lhsT matmul
  - D1 (device): h1|a1 = x @ [W1 | W1*att-blockdiag], emitted directly in
    T1 table-row byte layout (bf16 feats + fp8-packed tail + bf16 a_s).
  - D2 (device): layer-1 GAT aggregation per dst tile via dma_gather (one
    call per table zone per tile, exact num_idxs), windowed one-hot matmul
    segment softmax/sum, relu, on-core dense layer 2.
  - D3 (device): layer-2 aggregation, leaky_relu, classifier.
  - host: transpose/concat final logits.

Table rows (dma_gather requires a 256B multiple):
  L1: [248 feats bf16 | 8 feats fp8 | 4 a_s bf16]  (512B)
  L2: [120 feats bf16 | 8 feats fp8 | 4 a_s bf16]  (256B)
Score-weighting uses a pair-replicated broadcast AP so the DVE multiply
runs in 2x mode with no 64-wide score expansion.
"""

import os
import sys

import numpy as np

sys.path.insert(0, "/opt/trn_rl_repo")

import ml_dtypes  # noqa: E402

import concourse.bass as bass  # noqa: E402, F401
import concourse.mybir as mybir  # noqa: E402
import concourse.tile as tile  # noqa: E402
from concourse import bacc  # noqa: E402
from concourse.bass_utils import run_bass_kernel_spmd  # noqa: E402

# ---------------------------------------------------------------- constants
N = 50000
FIN = 256
H1, C1 = 4, 64
D1 = H1 * C1  # 256
H2, C2 = 4, 32
D2 = H2 * C2  # 128
NCLS = 7
NCORES = 8
P = 128
SHARD = 6272  # 49 tiles of 128 (>= ceil(50000/8))
NTILES = SHARD // P  # 49
NPAD = SHARD * NCORES  # 50176

LOWN = 32767  # nodes with src index in low table zone (0..32766)
TROWS = 50002  # 1 dummy + 32767 low + 1 dummy + 17233 high
HIGH_BASE = 32768  # table row of high-zone dummy

NEG_ATT = 0.2
NEG_ACT = 0.01
DUMMY_AS = -200.0
PADVAL = 500.0  # ds_rel value for pad/stale slots (one-hot all-zero)

TRACE = os.environ.get("GAT_TRACE", "0") == "1"
DDS = int(os.environ.get("GAT_DDS", "81920"))  # SWDGE scratch ring bytes (max that fits SBUF)
HOTBUFS = int(os.environ.get("GAT_BUFS", "4"))  # hot pool depth

NPDT = ml_dtypes.bfloat16
NPF8 = ml_dtypes.float8_e4m3
BF = mybir.dt.bfloat16
F32 = mybir.dt.float32
F8 = mybir.dt.float8e4

ELEM1 = 256  # L1 table slots (bf16 units): 248 bf16 + 8 fp8 + 4 bf16 a_s
NBF1 = 248  # leading bf16 feats in an L1 row
ELEM2 = 128  # L2 table slots (256B rows): 120 bf16 + 8 fp8 + 4 bf16 a_s
NBF2 = 120  # leading bf16 feats in an L2 row


def row_of_node(n):
    """table row for node index array n (vectorized)."""
    return np.where(n < LOWN, n + 1, n + 2)


def _r16(x):
    return (x + 15) // 16 * 16


def _r128(x):
    return (x + 127) // 128 * 128


# ---------------------------------------------------------------- host plan
class Plan:
    pass


def build_plan(edge_index):
    src = np.asarray(edge_index[0], dtype=np.int64)
    dst = np.asarray(edge_index[1], dtype=np.int64)
    loops = np.arange(N, dtype=np.int64)
    src = np.concatenate([src, loops])
    dst = np.concatenate([dst, loops])

    # degree-balanced node placement: deal each core's nodes into its 49x4
    # (tile, octant) buckets by descending degree (serpentine), so per-bucket
    # edge counts are near-equal across cores and region padding stays small
    deg = np.bincount(dst, minlength=N)
    posmap = np.full(N, -1, np.int64)         # node -> global position
    node_at_pos = np.full(NPAD, -1, np.int64)  # global position -> node
    NB = NTILES * 4
    for c in range(NCORES):
        ids = np.arange(c * SHARD, min((c + 1) * SHARD, N))
        order_ids = ids[np.argsort(-deg[ids], kind="stable")]
        for r in range(32):
            chunk = order_ids[r * NB : (r + 1) * NB]
            bseq = np.arange(NB) if r % 2 == 0 else np.arange(NB - 1, -1, -1)
            for i in range(len(chunk)):
                b = int(bseq[i])
                p = c * SHARD + (b // 4) * 128 + (b % 4) * 32 + r
                posmap[chunk[i]] = p
                node_at_pos[p] = chunk[i]

    dstp = posmap[dst]
    order = np.argsort(dstp, kind="stable")
    src = src[order].astype(np.int32)
    dst = dstp[order].astype(np.int32)
    starts = np.searchsorted(dst, np.arange(NPAD + 1))

    # per (core, tile, zone, octant): (idx_in_zone, dst_local) arrays
    reg = [[[[None] * 4 for _ in range(2)] for _ in range(NTILES)] for _ in range(NCORES)]
    for c in range(NCORES):
        for t in range(NTILES):
            g0 = c * SHARD + t * P
            e0, e1 = starts[g0], starts[g0 + P]
            s = src[e0:e1]
            dl = (dst[e0:e1] - g0).astype(np.int32)
            zlo = s < LOWN
            for z in range(2):
                zm = zlo if z == 0 else ~zlo
                sz = s[zm]
                dz = dl[zm]
                idxz = sz + 1 if z == 0 else sz - LOWN + 1
                for o in range(4):
                    om = (dz >= 32 * o) & (dz < 32 * (o + 1))
                    reg[c][t][z][o] = (idxz[om].astype(np.int32), dz[om])

    # shared region sizes (16-granular, max over cores)
    RSZ = np.zeros((NTILES, 2, 4), np.int32)
    for t in range(NTILES):
        for z in range(2):
            for o in range(4):
                m = max(len(reg[c][t][z][o][0]) for c in range(NCORES))
                RSZ[t, z, o] = _r16(m)
    SZ = RSZ.sum(axis=2)  # [NTILES, 2] exact zone slot counts
    # process tiles in descending slot-count order so the pipeline drains on
    # the cheapest tiles; the first FULLG processed (big) tiles gather full
    # chunks to flush stale SBUF in the G pool
    by_size = sorted(range(NTILES), key=lambda t: -int(SZ[t].sum()))
    FULLG = HOTBUFS + 1
    # smallest FULLG tiles first (fast pipeline fill; memset scrubs their
    # short G extent), then the rest descending so the drain is cheap too
    tile_order = by_size[-FULLG:][::-1] + by_size[: NTILES - FULLG]
    fullset = set(tile_order[:FULLG])
    NLO = np.array([_r128(SZ[t, 0]) if t in fullset else max(SZ[t, 0], 16)
                    for t in range(NTILES)], np.int64)
    NHI = np.array([_r128(SZ[t, 1]) if t in fullset else max(SZ[t, 1], 16)
                    for t in range(NTILES)], np.int64)
    CL = [_r128(NLO[t]) // 128 for t in range(NTILES)]
    CH = [_r128(NHI[t]) // 128 for t in range(NTILES)]
    C = [CL[t] + CH[t] for t in range(NTILES)]
    CMAX = max(C)

    # idx columns per (tile, zone)
    seg_off = []
    col = 0
    for t in range(NTILES):
        lo = col
        col += int(_r16(NLO[t]) // 16)
        hi = col
        col += int(_r16(NHI[t]) // 16)
        seg_off.append((lo, hi))
    COLS = col

    # block lists: per tile, list of (chunk, octant) with per-core data masks.
    # region slot layout within zone z: octants packed in order at 16-granular
    # offsets; chunk grid is per-zone (zone base chunk = 0 / CL[t]).
    blocks = []  # per tile: list of (g, o, s0, s1) zone-chunk-local slot range
    blk_off = [0]
    for t in range(NTILES):
        bl = []
        for z in range(2):
            zb_chunk = 0 if z == 0 else CL[t]
            nchunk = CL[t] if z == 0 else CH[t]
            rb = np.concatenate([[0], np.cumsum(RSZ[t, z])])
            for k in range(nchunk):
                s0, s1 = k * 128, (k + 1) * 128
                for o in range(4):
                    r0, r1 = int(rb[o]), int(rb[o + 1])
                    b0, b1 = max(s0, r0), min(s1, r1)
                    if b1 > b0:
                        bl.append((zb_chunk + k, o, b0 - s0, b1 - s0, z, k))
        blocks.append(bl)
        blk_off.append(blk_off[-1] + len(bl))
    NBLK_TOT = blk_off[-1]
    NBLKMAX = max(len(b) for b in blocks)

    # per-core arrays: idx16, ds_rel2, pt32
    idx16 = np.zeros((NCORES, 128, COLS), np.int16)
    ds_rel2 = np.full((NCORES, 128, 2 * NBLK_TOT), PADVAL, NPDT)
    pt32 = np.zeros((NCORES, 32, 128 * NBLK_TOT), NPF8)

    for c in range(NCORES):
        for t in range(NTILES):
            lo_col, hi_col = seg_off[t]
            # vd[z][slot] = dst_local or -1 pad, in packed zone-slot space
            vd = [np.full(int(_r128(NLO[t])), -1, np.int32),
                  np.full(int(_r128(NHI[t])), -1, np.int32)]
            for z in range(2):
                ncols = int(_r16(NLO[t] if z == 0 else NHI[t]) // 16)
                nslot = ncols * 16
                vi = np.zeros(nslot, np.int16)
                rb = 0
                for o in range(4):
                    idxz, dz = reg[c][t][z][o]
                    vi[rb: rb + len(idxz)] = idxz.astype(np.int16)
                    vd[z][rb: rb + len(dz)] = dz
                    rb += int(RSZ[t, z, o])
                seg = vi.reshape(-1, 16).T  # [16, nslot/16]
                cb = lo_col if z == 0 else hi_col
                for rep in range(8):
                    idx16[c, rep * 16: rep * 16 + 16, cb: cb + ncols] = seg
            # blocks -> ds_rel2 / pt32
            for bi, (g, o, p0, p1, z, k) in enumerate(blocks[t]):
                b = blk_off[t] + bi
                sl = vd[z][k * 128: (k + 1) * 128].copy()
                mask = np.zeros(128, bool)
                mask[p0:p1] = True
                mask &= sl >= 0
                rel = np.where(mask, sl - 32 * o, int(PADVAL)).astype(np.float32)
                ds_rel2[c, :, 2 * b] = rel.astype(NPDT)
                ds_rel2[c, :, 2 * b + 1] = rel.astype(NPDT)
                oh = (rel[None, :] == np.arange(32, dtype=np.float32)[:, None])
                pt32[c, :, b * 128: (b + 1) * 128] = oh.astype(NPF8)

    pl = Plan()
    pl.tile_order = tile_order
    pl.posmap, pl.node_at_pos = posmap, node_at_pos
    pl.src, pl.dst = src, dst
    pl.CL, pl.CH, pl.C, pl.CMAX = CL, CH, C, CMAX
    pl.NLO, pl.NHI = NLO, NHI
    pl.COLS, pl.seg_off = COLS, seg_off
    pl.blocks, pl.blk_off = blocks, blk_off
    pl.NBLK_TOT, pl.NBLKMAX = NBLK_TOT, NBLKMAX
    pl.idx16 = idx16
    pl.ds_rel2 = ds_rel2
    pl.pt32 = pt32
    return pl


# ------------------------------------------------------------ device builds
def build_d1():
    """dense: xT_shard.T @ [W | Wa] + [b | 0], emitted as L1 table rows.

    Outputs:
      out: [SHARD, 256] bf16 table rows [248 bf16 | 8 fp8 | 4 a_s bf16]
      adout: [SHARD, 4] bf16 a_d per node
    """
    EW = D1 + 8  # 264 dense output width
    nc = bacc.Bacc("TRN2", target_bir_lowering=False, debug=False, num_devices=NCORES)
    xT = nc.dram_tensor("xT", [FIN, SHARD], BF, kind="ExternalInput")
    wcat = nc.dram_tensor("wcat", [P, 2 * EW], BF, kind="ExternalInput")
    brow = nc.dram_tensor("brow", [1, EW], BF, kind="ExternalInput")
    onesc = nc.dram_tensor("onesc", [1, P], BF, kind="ExternalInput")
    out = nc.dram_tensor("out", [SHARD, ELEM1], BF, kind="ExternalOutput")
    adout = nc.dram_tensor("adout", [SHARD, 4], BF, kind="ExternalOutput")

    GB1 = 7
    with tile.TileContext(nc) as tc:
        with (
            tc.tile_pool(name="consts", bufs=1) as cpool,
            tc.tile_pool(name="lhs", bufs=3) as lpool,
            tc.tile_pool(name="res", bufs=3) as rpool,
            tc.tile_pool(name="ps", bufs=6, space="PSUM") as ppool,
        ):
            w_sb = cpool.tile([P, 2 * EW], BF)
            nc.sync.dma_start(w_sb, wcat.ap())
            b_sb = cpool.tile([1, EW], BF)
            nc.sync.dma_start(b_sb, brow.ap())
            ones_sb = cpool.tile([1, P], BF)
            nc.sync.dma_start(ones_sb, onesc.ap())
            for t0 in range(0, NTILES, GB1):
                ng = min(GB1, NTILES - t0)
                xs0 = lpool.tile([P, GB1 * P], BF, tag="xt0", name="xs0")
                nc.sync.dma_start(xs0[:, 0 : ng * P], xT.ap()[0:128, t0 * P : (t0 + ng) * P])
                xs1 = lpool.tile([P, GB1 * P], BF, tag="xt1", name="xs1")
                nc.sync.dma_start(xs1[:, 0 : ng * P], xT.ap()[128:256, t0 * P : (t0 + ng) * P])
                res = rpool.tile([P, GB1 * ELEM1], BF, tag="res", name="res")
                resf8 = res.bitcast(F8)
                adres = rpool.tile([P, GB1 * 4], BF, tag="adres", name="adres")
                for g in range(ng):
                    xt0 = xs0[:, g * P : (g + 1) * P]
                    xt1 = xs1[:, g * P : (g + 1) * P]
                    ps = ppool.tile([P, EW], F32, space="PSUM")
                    nc.tensor.matmul(ps, lhsT=xt0, rhs=w_sb[:, 0:EW], start=True, stop=False)
                    nc.tensor.matmul(ps, lhsT=xt1, rhs=w_sb[:, EW:], start=False, stop=False)
                    nc.tensor.matmul(ps, lhsT=ones_sb, rhs=b_sb, start=False, stop=True)
                    # emit table-row layout
                    nc.scalar.copy(out=res[:, g * ELEM1 : g * ELEM1 + NBF1], in_=ps[:, 0:NBF1])
                    nc.vector.tensor_copy(
                        out=resf8[:, g * 2 * ELEM1 + 2 * NBF1 : g * 2 * ELEM1 + 2 * NBF1 + 8],
                        in_=ps[:, NBF1:D1],
                    )
                    nc.vector.tensor_copy(
                        out=res[:, g * ELEM1 + 252 : g * ELEM1 + 256], in_=ps[:, D1 : D1 + 4]
                    )
                    nc.vector.tensor_copy(
                        out=adres[:, g * 4 : (g + 1) * 4], in_=ps[:, D1 + 4 : D1 + 8]
                    )
                oview = out.ap()[t0 * P : (t0 + ng) * P, :].rearrange(
                    "(g p) e -> p g e", p=P
                )
                nc.sync.dma_start(
                    oview, res[:, 0 : ng * ELEM1].rearrange("p (g e) -> p g e", e=ELEM1)
                )
                adview = adout.ap()[t0 * P : (t0 + ng) * P, :].rearrange(
                    "(g p) e -> p g e", p=P
                )
                nc.sync.dma_start(
                    adview, adres[:, 0 : ng * 4].rearrange("p (g e) -> p g e", e=4)
                )
    nc.compile()
    return nc


def build_agg(pl, layer):
    """Aggregation dispatch. layer=1: gather T1, produce T2 rows (h2|a2|ad2).
    layer=2: gather T2, produce classifier logits [8, 6272]."""
    if layer == 1:
        DFEAT, CH_, ELEM_T, NBF = D1, C1, ELEM1, NBF1  # 256, 64
    else:
        DFEAT, CH_, ELEM_T, NBF = D2, C2, ELEM2, NBF2  # 128, 32
    AS_SLOT = ELEM_T - 4
    HF = CH_ // 2  # feat pairs per head
    RW = DFEAT + 8  # rhs width: 4 heads x [CH_ feats | p p]
    HW = CH_ + 2  # per-head rhs block

    nc = bacc.Bacc("TRN2", target_bir_lowering=False, debug=False, num_devices=NCORES,
                   dynamic_dma_scratch_size=DDS)
    T = nc.dram_tensor("T", [TROWS, ELEM_T], BF, kind="ExternalInput")
    idx_d = nc.dram_tensor("idx", [128, pl.COLS], mybir.dt.int16, kind="ExternalInput")
    adoct_d = nc.dram_tensor("adoct", [32, NTILES * 16], BF, kind="ExternalInput")
    pt32_d = nc.dram_tensor("pt32", [32, 128 * pl.NBLK_TOT], F8, kind="ExternalInput")
    dsrel_d = nc.dram_tensor("dsrel", [128, 2 * pl.NBLK_TOT], BF, kind="ExternalInput")
    iota32_d = nc.dram_tensor("iota32", [P, 32], BF, kind="ExternalInput")
    zrow_d = nc.dram_tensor("zrow", [1, RW], BF, kind="ExternalInput")
    zcol_d = nc.dram_tensor("zcol", [1, P], BF, kind="ExternalInput")
    ident_d = nc.dram_tensor("ident", [P, P], BF, kind="ExternalInput")
    if layer == 1:
        W2W = D2 + 8  # 136
        w2cat_d = nc.dram_tensor("w2cat", [P, 2 * W2W], BF, kind="ExternalInput")
        b2row_d = nc.dram_tensor("b2row", [1, W2W], BF, kind="ExternalInput")
        ones_d = nc.dram_tensor("onesc", [1, P], BF, kind="ExternalInput")
        out = nc.dram_tensor("out", [((NTILES + 1) // 2) * P, 2 * ELEM2], BF, kind="ExternalOutput")
        adout = nc.dram_tensor("adout", [SHARD, 4], BF, kind="ExternalOutput")
    else:
        wl_d = nc.dram_tensor("wl", [P, 8], BF, kind="ExternalInput")
        bl_d = nc.dram_tensor("bl", [8, 1], F32, kind="ExternalInput")
        out = nc.dram_tensor("out", [8, SHARD], F32, kind="ExternalOutput")

    T_lo = T.ap()[0:HIGH_BASE, :]
    T_hi = T.ap()[HIGH_BASE:TROWS, :]

    with tile.TileContext(nc) as tc:
        with (
            tc.tile_pool(name="consts", bufs=1) as cpool,
            tc.tile_pool(name="gather", bufs=HOTBUFS + 1) as gpool,
            tc.tile_pool(name="onehot", bufs=HOTBUFS) as opool,
            tc.tile_pool(name="scores", bufs=HOTBUFS - 1) as spool,
            tc.tile_pool(name="small", bufs=3) as smpool,
            tc.tile_pool(name="psB", bufs=3, space="PSUM") as psB,   # scores
            tc.tile_pool(name="psC", bufs=3, space="PSUM") as psC,   # feat accum
            tc.tile_pool(name="psD", bufs=1, space="PSUM") as psD,   # transpose
            tc.tile_pool(name="psE", bufs=1, space="PSUM") as psE,   # dense2/cls
        ):
            # ---- constants / global loads
            qb = [0, 13, 25, 37, NTILES]  # tile-range quarters for idx load
            qcol = [pl.seg_off[b][0] if b < NTILES else pl.COLS for b in qb]
            idx_q = []
            for q in range(4):
                qt = cpool.tile([128, qcol[q + 1] - qcol[q]], mybir.dt.int16,
                                name=f"idxq{q}")
                nc.sync.dma_start(qt, idx_d.ap()[:, qcol[q] : qcol[q + 1]])
                idx_q.append(qt)

            def idx_slice(c0, c1):
                for q in range(4):
                    if c0 >= qcol[q] and c1 <= qcol[q + 1]:
                        return idx_q[q][:, c0 - qcol[q] : c1 - qcol[q]]
                raise AssertionError((c0, c1, qcol))
            adoct_sb = cpool.tile([32, NTILES * 16], BF)
            nc.sync.dma_start(adoct_sb, adoct_d.ap())
            dsrel_sb = cpool.tile([128, 2 * pl.NBLK_TOT], BF)
            nc.sync.dma_start(dsrel_sb, dsrel_d.ap())
            iota32_sb = cpool.tile([P, 32], BF)
            nc.sync.dma_start(iota32_sb, iota32_d.ap())
            zrow_sb = cpool.tile([1, RW], BF)
            nc.sync.dma_start(zrow_sb, zrow_d.ap())
            zcol_sb = cpool.tile([1, P], BF)
            nc.sync.dma_start(zcol_sb, zcol_d.ap())
            ident = cpool.tile([P, P], BF)
            nc.sync.dma_start(ident, ident_d.ap())
            if layer == 1:
                w2_sb = cpool.tile([P, 2 * W2W], BF)
                nc.sync.dma_start(w2_sb, w2cat_d.ap())
                b2_sb = cpool.tile([1, W2W], BF)
                nc.sync.dma_start(b2_sb, b2row_d.ap())
                ones_sb = cpool.tile([1, P], BF)
                nc.sync.dma_start(ones_sb, ones_d.ap())
            else:
                wl_sb = cpool.tile([P, 8], BF)
                nc.sync.dma_start(wl_sb, wl_d.ap())
                bl_sb = cpool.tile([8, 1], F32)
                nc.sync.dma_start(bl_sb, bl_d.ap())
                outbuf = cpool.tile([8, SHARD], F32)

            for pos, t in enumerate(pl.tile_order):
                C = pl.C[t]
                CL = pl.CL[t]
                CHh = pl.CH[t]
                lo_off, hi_off = pl.seg_off[t]
                blks = pl.blocks[t]
                nblk = len(blks)
                b0 = pl.blk_off[t]

                # ---- gather rows for this tile's edge slots (1 call/zone)
                Gfull = gpool.tile([128, pl.CMAX * ELEM_T], BF, tag="G", name="G")
                if pos < HOTBUFS + 1 and C < pl.CMAX:
                    # first use of this pool buffer: scrub the tail chunks a
                    # later (larger-C) tile would otherwise read uninitialized
                    nc.vector.memset(Gfull[:, C * ELEM_T :], 0.0)
                G = Gfull[:, : C * ELEM_T]
                G3 = G.rearrange("p (c e) -> p c e", e=ELEM_T)
                for (nidx, ncb, cb, off, src_ap) in (
                    (int(pl.NLO[t]), CL, 0, lo_off, T_lo),
                    (int(pl.NHI[t]), CHh, CL, hi_off, T_hi),
                ):
                    # device limit: at most 1024 rows (8 chunks) per call
                    for p0 in range(0, ncb, 8):
                        pc = min(8, ncb - p0)
                        nsub = min(nidx - p0 * 128, pc * 128)
                        if nsub <= 0:
                            continue
                        nc.gpsimd.dma_gather(
                            G3[:, cb + p0 : cb + p0 + pc, :],
                            src_ap,
                            idx_slice(off + p0 * 8, off + p0 * 8 + _r16(nsub) // 16),
                            nsub,
                            nsub,
                            ELEM_T,
                        )

                # ---- one-hots: Pm32 on DVE, PT32 from DRAM
                Pm32 = opool.tile([128, pl.NBLKMAX * 32], BF, tag="Pm", name="Pm")[:, : nblk * 32]
                pm4 = Pm32.rearrange("p (b r two) -> p b r two", r=16, two=2)
                dsb = dsrel_sb[:, 2 * b0 : 2 * (b0 + nblk)].rearrange(
                    "p (b two) -> p b two", two=2
                ).unsqueeze(2).to_broadcast([128, nblk, 16, 2])
                iob = iota32_sb.rearrange("p (r two) -> p r two", two=2).unsqueeze(
                    1
                ).to_broadcast([128, nblk, 16, 2])
                nc.vector.tensor_tensor(out=pm4, in0=dsb, in1=iob, op=mybir.AluOpType.is_equal)
                PT32 = opool.tile([32, pl.NBLKMAX * 128], F8, tag="PT", name="PT")[:, : nblk * 128]
                nc.sync.dma_start(PT32, pt32_d.ap()[:, b0 * 128 : (b0 + nblk) * 128])

                # ---- scores: pssc[slot, c*4+h] = a_s + a_d via PSUM matmuls
                pssc = psB.tile([P, pl.CMAX * 4], F32, tag="pssc", space="PSUM")
                # group blocks by chunk for start/stop bookkeeping
                chunk_blocks = [[] for _ in range(C)]
                for bi, blk in enumerate(blks):
                    chunk_blocks[blk[0]].append((bi, blk))
                for g in range(C):
                    nc.tensor.matmul(
                        pssc[:, g * 4 : (g + 1) * 4],
                        lhsT=ident,
                        rhs=G3[:, g, AS_SLOT : AS_SLOT + 4],
                        start=True, stop=(len(chunk_blocks[g]) == 0),
                        skip_group_check=True,
                    )
                    for j, (bi, (gg, o, p0, p1, z, k)) in enumerate(chunk_blocks[g]):
                        nc.tensor.matmul(
                            pssc[:, g * 4 : (g + 1) * 4],
                            lhsT=PT32[:, bi * 128 : (bi + 1) * 128],
                            rhs=adoct_sb[:, (t * 4 + o) * 4 : (t * 4 + o) * 4 + 4],
                            start=False, stop=(j == len(chunk_blocks[g]) - 1),
                            skip_group_check=True,
                        )

                # ---- pexp2 = exp(leaky(pssc)) = max(exp(x), exp(0.2x)),
                # pair-replicated; ACT reads PSUM directly, DVE max on bf16
                e_b = pssc[:, : C * 4].rearrange("p (c h) -> p c h", h=4).unsqueeze(
                    3
                ).to_broadcast([128, C, 4, 2])
                pexpa = spool.tile([128, pl.CMAX * 8], BF, tag="pexpa", name="pexpa")[:, : C * 8]
                nc.scalar.activation(
                    out=pexpa.rearrange("p (c h two) -> p c h two", h=4, two=2),
                    in_=e_b, func=mybir.ActivationFunctionType.Exp,
                )
                pexpb = spool.tile([128, pl.CMAX * 8], BF, tag="pexpb", name="pexpb")[:, : C * 8]
                nc.scalar.activation(
                    out=pexpb.rearrange("p (c h two) -> p c h two", h=4, two=2),
                    in_=e_b, func=mybir.ActivationFunctionType.Exp, scale=NEG_ATT,
                )
                pexp2 = spool.tile([128, pl.CMAX * 8], BF, tag="pexp2", name="pexp2")[:, : C * 8]
                nc.vector.tensor_tensor(
                    out=pexp2, in0=pexpa, in1=pexpb, op=mybir.AluOpType.max
                )
                pexp2_4 = pexp2.rearrange("p (c h two) -> p c h two", h=4, two=2)

                # ---- weighted rhs: RA chunk = 4 heads x [CH_ feats*p | p p]
                RA = spool.tile([128, pl.CMAX * RW], BF, tag="RA", name="RA")[:, : C * RW]
                RAh = RA.rearrange("p (c e) -> p c e", e=RW).rearrange(
                    "p c (h w) -> p c h w", w=HW
                )
                # main multiply in 2x mode via pair-replicated pexp2 broadcast;
                # all views fold (c, h) -> g to stay within 3 free dims
                feat_pair = G.rearrange("p (g f) -> p g f", f=CH_).rearrange(
                    "p g (r two) -> p g r two", two=2
                )
                ra_pair = RA.rearrange("p (g w) -> p g w", w=HW)[:, :, 0:CH_].rearrange(
                    "p g (r two) -> p g r two", two=2
                )
                pexp_b = pexp2.rearrange("p (g two) -> p g two", two=2).unsqueeze(
                    2
                ).to_broadcast([128, C * 4, HF, 2])
                nc.vector.tensor_tensor(
                    out=ra_pair, in0=feat_pair, in1=pexp_b, op=mybir.AluOpType.mult
                )
                # fix up the 8 fp8-packed feats (displaced to head-3 tail)
                G8 = G.bitcast(F8).rearrange("p (c e) -> p c e", e=2 * ELEM_T)
                f8pair = G8[:, :, 2 * NBF : 2 * NBF + 8].rearrange(
                    "p c (r two) -> p c r two", two=2
                )
                ra_fix = RAh[:, :, 3, CH_ - 8 : CH_].rearrange(
                    "p c (r two) -> p c r two", two=2
                )
                pfix_b = pexp2_4[:, :, 3:4, :].to_broadcast([128, C, 4, 2])
                nc.vector.tensor_tensor(
                    out=ra_fix, in0=f8pair, in1=pfix_b, op=mybir.AluOpType.mult
                )
                # p pair into the per-head tail columns (ACT, to unblock
                # DVE's next-chunk multiply)
                nc.scalar.copy(out=RAh[:, :, :, CH_ : CH_ + 2], in_=pexp2_4)

                # ---- accumulate: psout via windowed one-hot matmuls
                psout = psC.tile([P, RW], F32, tag="psout", space="PSUM")
                nc.tensor.matmul(psout, lhsT=zcol_sb, rhs=zrow_sb, start=True,
                                 stop=False, skip_group_check=True)
                for bi, (g, o, p0, p1, z, k) in enumerate(blks):
                    nc.tensor.matmul(
                        psout[32 * o : 32 * o + 32, :],
                        lhsT=Pm32[:, bi * 32 : (bi + 1) * 32],
                        rhs=RA[:, g * RW : (g + 1) * RW],
                        start=False, stop=(bi == nblk - 1),
                        skip_group_check=True,
                        tile_position=(0, 32 * o),
                    )

                # ---- normalize: o1[p, h, f] = num / den
                psout_h = psout.rearrange("p (h w) -> p h w", w=HW)
                recip = smpool.tile([P, 4], F32, tag="recip")
                nc.vector.reciprocal(recip, psout_h[:, :, CH_])
                o1 = smpool.tile([P, DFEAT], F32, tag="o1")
                nc.vector.tensor_tensor(
                    out=o1.rearrange("p (h f) -> p h f", f=CH_),
                    in0=psout_h[:, :, 0:CH_],
                    in1=recip.unsqueeze(2).to_broadcast([P, 4, CH_]),
                    op=mybir.AluOpType.mult,
                )

                if layer == 1:
                    # relu -> r1 (bf16), transpose, dense-2, +b2, write T2 rows
                    r1 = smpool.tile([P, DFEAT], BF, tag="r1")
                    nc.scalar.activation(
                        out=r1, in_=o1, func=mybir.ActivationFunctionType.Relu
                    )
                    pse = psE.tile([P, W2W], F32, tag="pse", space="PSUM")
                    for h in range(2):
                        pst = psD.tile([P, P], BF, tag="pst", space="PSUM")
                        nc.tensor.transpose(pst, r1[:, h * P : (h + 1) * P], ident)
                        r1T = smpool.tile([P, P], BF, tag="r1T")
                        nc.scalar.copy(out=r1T, in_=pst)
                        nc.tensor.matmul(
                            pse,
                            lhsT=r1T,
                            rhs=w2_sb[:, h * W2W : (h + 1) * W2W],
                            start=(h == 0), stop=False,
                        )
                    nc.tensor.matmul(pse, lhsT=ones_sb, rhs=b2_sb, start=False, stop=True)
                    if pos % 2 == 0:
                        t2pair = smpool.tile([P, 2 * ELEM2], BF, tag="t2pair", name="t2pair")
                    t2row = t2pair[:, (pos % 2) * ELEM2 : (pos % 2 + 1) * ELEM2]
                    t2f8 = t2pair.bitcast(F8)[
                        :, (pos % 2) * 2 * ELEM2 : (pos % 2 + 1) * 2 * ELEM2
                    ]
                    nc.scalar.copy(out=t2row[:, 0:NBF2], in_=pse[:, 0:NBF2])
                    nc.scalar.copy(
                        out=t2f8[:, 2 * NBF2 : 2 * NBF2 + 8], in_=pse[:, NBF2:D2]
                    )
                    nc.scalar.copy(
                        out=t2row[:, ELEM2 - 4 : ELEM2], in_=pse[:, D2 : D2 + 4]
                    )
                    adrow = smpool.tile([P, 4], BF, tag="adrow")
                    nc.scalar.copy(out=adrow, in_=pse[:, D2 + 4 : D2 + 8])
                    if pos % 2 == 1 or pos == NTILES - 1:
                        nc.sync.dma_start(
                            out.ap()[(pos // 2) * P : (pos // 2 + 1) * P, :], t2pair
                        )
                    nc.sync.dma_start(adout.ap()[t * P : (t + 1) * P, :], adrow)
                else:
                    # leaky(0.01) -> transpose -> classifier -> outbuf
                    o2 = smpool.tile([P, DFEAT], BF, tag="o2")
                    nc.vector.scalar_tensor_tensor(
                        out=o2, in0=o1, scalar=NEG_ACT, in1=o1,
                        op0=mybir.AluOpType.mult, op1=mybir.AluOpType.max,
                    )
                    pst = psD.tile([P, P], BF, tag="pst", space="PSUM")
                    nc.tensor.transpose(pst, o2, ident)
                    o2T = smpool.tile([P, P], BF, tag="o2T")
                    nc.scalar.copy(out=o2T, in_=pst)
                    psc = psE.tile([8, P], F32, tag="psc", space="PSUM")
                    nc.tensor.matmul(psc, lhsT=wl_sb, rhs=o2T, start=True, stop=True)
                    nc.scalar.activation(
                        out=outbuf[:, t * P : (t + 1) * P], in_=psc,
                        func=mybir.ActivationFunctionType.Identity, bias=bl_sb,
                    )
            if layer == 2:
                nc.sync.dma_start(out.ap(), outbuf)
    nc.compile()
    return nc


# ------------------------------------------------------------------ helpers
def _wcat(W, att_src, att_dst, heads, chan):
    """[W | W@blockdiag(att_src) | W@blockdiag(att_dst)] -> [K, D+8]"""
    K, Dh = W.shape
    wa_s = np.zeros((K, heads), np.float32)
    wa_d = np.zeros((K, heads), np.float32)
    for h in range(heads):
        wa_s[:, h] = W[:, h * chan : (h + 1) * chan] @ att_src[h]
        wa_d[:, h] = W[:, h * chan : (h + 1) * chan] @ att_dst[h]
    return np.concatenate([W, wa_s, wa_d], axis=1).astype(np.float32)


def _chunk_major(Wfull, width=None):
    """[256, E] -> [128, 2*width] (K-chunk-major for SBUF, zero-padded)"""
    e = Wfull.shape[1]
    width = width or e
    out = np.zeros((128, 2 * width), Wfull.dtype)
    out[:, 0:e] = Wfull[0:128, :]
    out[:, width : width + e] = Wfull[128:256, :]
    return out


def _make_table(rows_bf, elem, node_at_pos):
    """Assemble a gather table [TROWS, elem] bf16 from device row-layout out."""
    Tb = np.zeros((TROWS, elem), NPDT)
    valid = node_at_pos >= 0
    Tb[row_of_node(node_at_pos[valid])] = rows_bf[valid]
    Tb[0, elem - 4 : elem] = DUMMY_AS
    Tb[HIGH_BASE, elem - 4 : elem] = DUMMY_AS
    return Tb


def _ad_oct(a_d):
    """[NPAD, 4] padded a_d -> per-core [32, NTILES*16] bf16 (octant-sliced)."""
    out = np.zeros((NCORES, 32, NTILES * 16), np.float32)
    for c in range(NCORES):
        blk = a_d[c * SHARD : (c + 1) * SHARD].reshape(NTILES, 4, 32, 4)  # t, o, p, h
        out[c] = blk.transpose(2, 0, 1, 3).reshape(32, NTILES * 16)
    return out.astype(NPDT)


_CACHE = {}


def _run(nc, in_maps, tag):
    trace = TRACE
    if trace:
        try:
            from antenv.axon_hooks import get_axon_ntff_profile_hook  # noqa: F401
        except ImportError:
            trace = False
    res = run_bass_kernel_spmd(nc, in_maps, core_ids=list(range(NCORES)), trace=trace)
    if trace and res.exec_time_ns:
        print(f"[{tag}] exec_time_ns = {res.exec_time_ns}", file=sys.stderr)
        _CACHE.setdefault("times", {})[tag] = res.exec_time_ns
    return res.results


# -------------------------------------------------------------------- main
def kernel(x, edge_index, W1, att_src1, att_dst1, b1, W2, att_src2, att_dst2, b2, Wl, bl):
    x = np.asarray(x, np.float32)
    W1 = np.asarray(W1, np.float32)
    W2 = np.asarray(W2, np.float32)
    Wl = np.asarray(Wl, np.float32)
    b1 = np.asarray(b1, np.float32)
    b2 = np.asarray(b2, np.float32)
    bl = np.asarray(bl, np.float32)
    att_src1 = np.asarray(att_src1, np.float32)
    att_dst1 = np.asarray(att_dst1, np.float32)
    att_src2 = np.asarray(att_src2, np.float32)
    att_dst2 = np.asarray(att_dst2, np.float32)

    pl = build_plan(np.asarray(edge_index))

    iota32 = np.arange(32, dtype=np.float32)[None, :].repeat(P, axis=0).astype(NPDT)
    ident = np.eye(P, dtype=np.float32).astype(NPDT)
    zcol = np.zeros((1, P), NPDT)

    # ---------------- D1: dense layer-1 (emits T1 row layout directly)
    w1cat = _wcat(W1, att_src1, att_dst1, H1, C1)  # [256, 264]
    xT = np.zeros((FIN, NPAD), np.float32)
    valid = pl.node_at_pos >= 0
    xT[:, valid] = x.T[:, pl.node_at_pos[valid]]
    d1_in = []
    for c in range(NCORES):
        d1_in.append({
            "xT": xT[:, c * SHARD : (c + 1) * SHARD].astype(NPDT),
            "wcat": _chunk_major(w1cat).astype(NPDT),
            "brow": np.concatenate([b1, np.zeros(8, np.float32)])[None, :].astype(NPDT),
            "onesc": np.ones((1, P), NPDT),
        })
    if "d1" not in _CACHE:
        _CACHE["d1"] = build_d1()
    r1 = _run(_CACHE["d1"], d1_in, "d1")
    rows_bf = np.concatenate([r["out"] for r in r1], axis=0)  # [NPAD, 256] bf16
    ad_bf = np.concatenate([r["adout"] for r in r1], axis=0)  # [NPAD, 4] bf16

    # ---------------- host: assemble T1 + a_d input (position-ordered)
    T1 = _make_table(rows_bf, ELEM1, pl.node_at_pos)
    ad1 = ad_bf.astype(np.float32)
    ad1[~valid] = 0.0  # pad positions: NaN a_d would poison window matmuls
    ad1_in = _ad_oct(ad1)

    # ---------------- D2: layer-1 aggregation + dense layer-2
    w2cat = _wcat(W2, att_src2, att_dst2, H2, C2)  # [256, 136]
    W2W = D2 + 8
    d2_in = []
    for c in range(NCORES):
        d2_in.append({
            "T": T1,
            "idx": pl.idx16[c],
            "adoct": ad1_in[c],
            "pt32": pl.pt32[c],
            "dsrel": pl.ds_rel2[c],
            "iota32": iota32,
            "zrow": np.zeros((1, D1 + 8), NPDT),
            "zcol": zcol,
            "ident": ident,
            "w2cat": _chunk_major(w2cat, W2W).astype(NPDT),
            "b2row": np.concatenate([b2, np.zeros(8, np.float32)])[None, :].astype(NPDT),
            "onesc": np.ones((1, P), NPDT),
        })
    key = ("d2", pl.COLS, pl.NBLK_TOT, tuple(pl.C))
    if key not in _CACHE:
        _CACHE[key] = build_agg(pl, 1)
    r2 = _run(_CACHE[key], d2_in, "d2")
    rows2_bf = np.zeros((NPAD, ELEM2), NPDT)
    npair = (NTILES + 1) // 2
    for c in range(NCORES):
        blk = r2[c]["out"]  # [npair*P, 2*ELEM2]: pair q = order pos 2q, 2q+1
        seq = blk.reshape(npair, P, 2, ELEM2).transpose(0, 2, 1, 3).reshape(
            npair * 2, P, ELEM2
        )
        for p_i, t_i in enumerate(pl.tile_order):
            rows2_bf[c * SHARD + t_i * P : c * SHARD + (t_i + 1) * P] = seq[p_i]
    ad2_bf = np.concatenate([r["adout"] for r in r2], axis=0)  # [NPAD, 4] bf16

    # ---------------- host: assemble T2 + a_d input
    T2 = _make_table(rows2_bf, ELEM2, pl.node_at_pos)
    ad2 = ad2_bf.astype(np.float32)
    ad2[~valid] = 0.0
    np.nan_to_num(ad2, copy=False)
    ad2_in = _ad_oct(ad2)

    # ---------------- D3: layer-2 aggregation + classifier
    wl8 = np.zeros((P, 8), np.float32)
    wl8[:, 0:NCLS] = Wl
    bl8 = np.zeros((8, 1), np.float32)
    bl8[0:NCLS, 0] = bl
    d3_in = []
    for c in range(NCORES):
        d3_in.append({
            "T": T2,
            "idx": pl.idx16[c],
            "adoct": ad2_in[c],
            "pt32": pl.pt32[c],
            "dsrel": pl.ds_rel2[c],
            "iota32": iota32,
            "zrow": np.zeros((1, D2 + 8), NPDT),
            "zcol": zcol,
            "ident": ident,
            "wl": wl8.astype(NPDT),
            "bl": bl8,
        })
    key3 = ("d3", pl.COLS, pl.NBLK_TOT, tuple(pl.C))
    if key3 not in _CACHE:
        _CACHE[key3] = build_agg(pl, 2)
    r3 = _run(_CACHE[key3], d3_in, "d3")

    out = np.zeros((N, NCLS), np.float32)
    for c in range(NCORES):
        blk = r3[c]["out"]  # [8, SHARD] in position order
        nap = pl.node_at_pos[c * SHARD : (c + 1) * SHARD]
        v = nap >= 0
        out[nap[v]] = blk[0:NCLS, v].T
    return out
